# revision 1
# baseline (speedup 1.0000x reference)
"""Causal single-head attention on 8 Trainium2 NeuronCores.

Problem: B=4, S=2048, D_IN=1024, D_OUT=64 (fp32).
  Q = Xq @ Wq; K = Xk @ Wk; V = Xv @ Wv
  out = softmax(mask(Q K^T / 8)) @ V

Sharding: 8 cores = 4 batches x 2 interleaved query-block sets.
Core c handles batch b = c//2 and query blocks {128*(2t+h) : t in 0..7},
h = c%2.  The interleave balances causal work (64 vs 72 k-tile pairs).

Per-core pipeline (all SPMD-uniform; h enters only via host-side data):
  1. Stream X tiles, PE-transpose to get d-on-partitions X^T (bf16), and
     project with W stationary:  Q^T/8, K^T, V^T  [64, S] bf16.
  2. Build V1 = [V | 1] natural layout via small PE transposes of V^T.
  3. Attention with scores TRANSPOSED (keys on partitions):
       scoresT[k,q] = K Q^T  -> exp (no max-subtract; scores are O(1))
       -> causal mask via host-provided 0/1 tiles on the <=2 boundary
          k-tiles per q-block -> AV^T[e',q] = V1^T expT accumulated over
          k-tiles in PSUM (column 64 = softmax denominator).
  4. Transpose AV^T back, divide by denominator, DMA out.
"""

import os
import numpy as np
import ml_dtypes

import concourse.bass as bass
import concourse.mybir as mybir
import concourse.tile as tile
from concourse.bass_utils import run_bass_kernel_spmd
from concourse.masks import make_identity
from concourse.vector_clock import ScopedClock

# ---------------------------------------------------------------------------
# Workaround: the walrus in this container rejects Tile's end-of-kernel drain
# when it carries >1 sem wait ("Too many sync wait commands").  Split the
# waits across single-wait SP NOPs placed just before the drain.
# ---------------------------------------------------------------------------


def _patched_drain_and_barrier(self, tick_clock, wait_clock):
    nc = self.nc
    collector = nc.sync.nop(nofuse=True)
    wait_clock.add_sem_waits(
        collector.ins, ScopedClock({None: tick_clock.global_clock})
    )
    si = collector.ins.sync_info
    waits = list(si.on_wait or []) if si is not None else []
    if si is not None:
        si.on_wait = waits[:1]
    for w in waits[1:]:
        n = nc.sync.nop(nofuse=True)
        nsi = n.ins.sync_info
        if nsi is None:
            n.ins.sync_info = mybir.SyncInfo(on_wait=[w], on_update=[])
        else:
            nsi.on_wait = [w]
    nc.sync.drain()
    nc.all_engine_barrier()
    assert self.sems is not None
    popped = nc._tile_sem_poison_stack.pop()
    assert popped is self._sem_poison
    nc.clear_and_free_semaphores(list(self.sems.allocated().values()))
    nc.all_engine_barrier()


tile.TileContext._drain_and_barrier = _patched_drain_and_barrier


def _split_sync_waits(nc, limit=1):
    """The nix walrus allows only `limit` sem waits per instruction; hoist
    extras onto same-engine NOPs placed immediately before the instruction."""
    ctr = [0]
    for fn in nc.m.functions:
        for bb in fn.blocks:
            out_list = []
            changed = False
            for inst in bb.instructions:
                si = inst.sync_info
                waits = list(si.on_wait) if si is not None and si.on_wait else []
                if len(waits) > limit:
                    keep = waits[-limit:]
                    for w in waits[:-limit]:
                        ctr[0] += 1
                        nop = mybir.InstNoOp(
                            name=f"waitsplit-{ctr[0]}",
                            engine=inst.engine,
                            ins=[],
                            outs=[],
                            sync_info=mybir.SyncInfo(on_wait=[w], on_update=[]),
                        )
                        out_list.append(nop)
                    si.on_wait = keep
                    changed = True
                out_list.append(inst)
            if changed:
                bb.instructions = out_list

# ---------------------------------------------------------------------------

B, S, D, E = 4, 2048, 1024, 64
SC = S // 2          # query rows per core
NT = SC // 128       # 8 local query blocks
NKT = S // 128       # 16 k-tiles
ND = D // 128        # 8 d-tiles
GROUP = 512          # s rows per streaming group (4 s-tiles)

F32 = mybir.dt.float32
BF16 = mybir.dt.bfloat16
EXP = mybir.ActivationFunctionType.Exp

# Unique-signature tag: the jax/neuron compile cache keys collide for
# same-signature modules, so every kernel variant carries a dummy input
# whose shape encodes the variant id.
KERNEL_UID = 91


def _build_nc(loop_reps=None, split=True, timing_mode=False, use_cc=False, uid=KERNEL_UID):
    nc = bass.Bass()

    kv_rows = SC if use_cc else S
    if timing_mode:
        # Internal (device-zeroed) X tensors: shrinks per-call transfer so the
        # K-rep wall-clock slope resolves the kernel's true exec time.
        xq = nc.dram_tensor("xq", (SC, D), F32)
        xk = nc.dram_tensor("xk", (kv_rows, D), F32)
        xv = nc.dram_tensor("xv", (kv_rows, D), F32)
    else:
        xq = nc.dram_tensor("xq", (SC, D), F32, kind="ExternalInput")
        xk = nc.dram_tensor("xk", (kv_rows, D), F32, kind="ExternalInput")
        xv = nc.dram_tensor("xv", (kv_rows, D), F32, kind="ExternalInput")
    wq = nc.dram_tensor("wq", (D, E), F32, kind="ExternalInput")
    wk = nc.dram_tensor("wk", (D, E), F32, kind="ExternalInput")
    wv = nc.dram_tensor("wv", (D, E), F32, kind="ExternalInput")
    masks = nc.dram_tensor("masks", (NT, 2, 128, 128), BF16, kind="ExternalInput")
    nc.dram_tensor("vtag", (1, uid), F32, kind="ExternalInput")
    out = nc.dram_tensor("out", (SC, E), F32, kind="ExternalOutput")

    with tile.TileContext(nc) as tc:
        with (
            tc.tile_pool(name="const", bufs=1) as cpool,
            tc.tile_pool(name="stage", bufs=5) as spool,
            tc.tile_pool(name="xt", bufs=3) as xtpool,
            tc.tile_pool(name="exp", bufs=3) as epool,
            tc.tile_pool(name="fin", bufs=2) as fpool,
            tc.tile_pool(name="ps_tp", bufs=3, space="PSUM") as ps_tp,
            tc.tile_pool(name="ps_proj", bufs=1, space="PSUM") as ps_proj,
            tc.tile_pool(name="ps_sc", bufs=2, space="PSUM") as ps_sc,
            tc.tile_pool(name="ps_av", bufs=2, space="PSUM") as ps_av,
            tc.tile_pool(name="dram", bufs=1, space="DRAM") as dpool,
        ):
            # ---- one-time constants ----
            ident_f = cpool.tile([128, 128], F32, tag="ident_f")
            make_identity(nc, ident_f)
            ident_b = cpool.tile([128, 128], BF16, tag="ident_b")
            make_identity(nc, ident_b)

            w_sb = {}
            for name, w in (("q", wq), ("k", wk), ("v", wv)):
                t = cpool.tile([128, ND, E], BF16, tag=f"w_{name}")
                nc.gpsimd.dma_start(
                    out=t[:], in_=w[:, :].rearrange("(a p) e -> p a e", p=128)
                )
                w_sb[name] = t

            mask_sb = cpool.tile([128, NT, 2, 128], BF16, tag="mask")
            nc.sync.dma_start(
                out=mask_sb[:], in_=masks[:, :, :, :].rearrange("t j p q -> p t j q")
            )

            def emit_body():
                qt = cpool.tile([E, SC], BF16, tag="qt")
                kt_sb = cpool.tile([E, S], BF16, tag="kt")
                vt = cpool.tile([E, S], BF16, tag="vt")
                v1 = cpool.tile([128, NKT, E + 1], BF16, tag="v1")
                out_sb = cpool.tile([128, NT, E], F32, tag="out_sb")

                # ---- phase 1: cast-load + transpose + project ----
                def load_project(x_h, n_rows, w_tile, dst, scale):
                    for g in range(n_rows // GROUP):
                        stage = spool.tile([128, 4, D], BF16, tag="stage")
                        nc.gpsimd.dma_start(
                            out=stage[:],
                            in_=x_h[g * GROUP : (g + 1) * GROUP, :].rearrange(
                                "(st p) d -> p st d", p=128
                            ),
                        )
                        xt_t = xtpool.tile([128, ND, GROUP], BF16, tag="xt")
                        for dt in range(ND):
                            ps = ps_tp.tile([128, GROUP], BF16, tag="tp")
                            for st in range(4):
                                nc.tensor.transpose(
                                    ps[:, st * 128 : (st + 1) * 128],
                                    stage[:, st, dt * 128 : (dt + 1) * 128],
                                    ident_b[:],
                                )
                            nc.vector.tensor_copy(out=xt_t[:, dt, :], in_=ps[:])
                        pps = ps_proj.tile([E, GROUP], F32, tag="proj")
                        for dt in range(ND):
                            nc.tensor.matmul(
                                pps[:],
                                w_tile[:, dt, :],
                                xt_t[:, dt, :],
                                start=(dt == 0),
                                stop=(dt == ND - 1),
                            )
                        if scale is None:
                            nc.scalar.copy(
                                out=dst[:, g * GROUP : (g + 1) * GROUP], in_=pps[:]
                            )
                        else:
                            nc.scalar.mul(
                                dst[:, g * GROUP : (g + 1) * GROUP], pps[:], scale
                            )

                def gather_pair(local, full, tag):
                    src_d = dpool.tile([E, SC], BF16, tag=f"cc_src_{tag}")
                    dst_d = dpool.tile([2, E, SC], BF16, tag=f"cc_dst_{tag}")
                    nc.sync.dma_start(out=src_d[:], in_=local[:])
                    nc.gpsimd.collective_compute(
                        "AllGather",
                        mybir.AluOpType.bypass,
                        replica_groups=[[0, 1], [2, 3], [4, 5], [6, 7]],
                        ins=[src_d[:]],
                        outs=[dst_d[:]],
                    )
                    nc.sync.dma_start(
                        out=full[:].rearrange("e (r s) -> e r s", r=2),
                        in_=dst_d[:].rearrange("r e s -> e r s"),
                    )

                if use_cc:
                    kt_half = cpool.tile([E, SC], BF16, tag="kt_half")
                    vt_half = cpool.tile([E, SC], BF16, tag="vt_half")
                    load_project(xk, SC, w_sb["k"], kt_half, None)
                    gather_pair(kt_half, kt_sb, "k")
                    load_project(xv, SC, w_sb["v"], vt_half, None)
                    gather_pair(vt_half, vt, "v")
                    load_project(xq, SC, w_sb["q"], qt, 1.0 / np.sqrt(E))
                else:
                    load_project(xq, SC, w_sb["q"], qt, 1.0 / np.sqrt(E))
                    load_project(xk, S, w_sb["k"], kt_sb, None)
                    load_project(xv, S, w_sb["v"], vt, None)

                # ---- phase 2: V natural + ones column ----
                nc.vector.memset(v1[:], 1.0)
                for kti in range(NKT):
                    ps = ps_tp.tile([128, 128], BF16, tag="tp")
                    nc.tensor.transpose(
                        ps[:, 0:E],
                        vt[:, kti * 128 : (kti + 1) * 128],
                        ident_b[0:E, 0:E],
                    )
                    nc.vector.tensor_copy(out=v1[:, kti, 0:E], in_=ps[:, 0:E])

                # ---- phase 3: attention ----
                av0 = ps_av.tile([E + 1, 512], F32, tag="av")
                av1 = ps_av.tile([E + 1, 512], F32, tag="av")
                for kti in range(NKT):
                    t0 = kti // 2
                    expt = epool.tile([128, SC], BF16, tag="expt")
                    for c in (0, 1):
                        t_start = max(t0, 4 * c)
                        t_end = 4 * c + 4
                        if t_start >= t_end:
                            continue
                        width = (t_end - t_start) * 128
                        sps = ps_sc.tile([128, 512], F32, tag="sc")
                        nc.tensor.matmul(
                            sps[:, 0:width],
                            kt_sb[:, kti * 128 : (kti + 1) * 128],
                            qt[:, t_start * 128 : t_end * 128],
                            start=True,
                            stop=True,
                        )
                        nc.scalar.activation(
                            expt[:, t_start * 128 : t_end * 128], sps[:, 0:width], EXP
                        )
                    # causal mask on the diagonal/boundary tile (t == t0 only)
                    nc.vector.tensor_mul(
                        expt[:, t0 * 128 : (t0 + 1) * 128],
                        expt[:, t0 * 128 : (t0 + 1) * 128],
                        mask_sb[:, t0, kti - 2 * t0, :],
                    )
                    # zero stale columns of skipped t's inside a live AV chunk
                    if 0 < t0 < 4:
                        nc.gpsimd.memset(expt[:, 0 : t0 * 128], 0.0)
                    elif 4 < t0 < 8:
                        nc.gpsimd.memset(expt[:, 512 : t0 * 128], 0.0)
                    if t0 < 4:
                        nc.tensor.matmul(
                            av0[:],
                            v1[:, kti, :],
                            expt[:, 0:512],
                            start=(kti == 0),
                            stop=(kti == 7),
                            skip_group_check=True,
                        )
                    nc.tensor.matmul(
                        av1[:],
                        v1[:, kti, :],
                        expt[:, 512:SC],
                        start=(kti == 0),
                        stop=(kti == NKT - 1),
                        skip_group_check=True,
                    )

                # ---- phase 4: divide + transpose back + store ----
                avsb = fpool.tile([E + 1, SC], F32, tag="avsb", bufs=1)
                nc.vector.tensor_copy(out=avsb[:, 0:512], in_=av0[:])
                nc.vector.tensor_copy(out=avsb[:, 512:SC], in_=av1[:])
                for t in range(NT):
                    nps = ps_tp.tile([128, 128], F32, tag="tp")
                    nc.tensor.transpose(
                        nps[:, 0 : E + 1],
                        avsb[:, t * 128 : (t + 1) * 128],
                        ident_f[0 : E + 1, 0 : E + 1],
                    )
                    rec = fpool.tile([128, 1], F32, tag="rec")
                    nc.vector.reciprocal(rec[:], nps[:, E : E + 1])
                    nc.vector.tensor_scalar_mul(out_sb[:, t, :], nps[:, 0:E], rec[:])

                nc.sync.dma_start(
                    out=out[:, :].rearrange("(t p) e -> p t e", p=128), in_=out_sb[:]
                )


            if timing_mode:
                zt = cpool.tile([128, 1024], F32, tag="zt")
                nc.vector.memset(zt[:], 0.0)
                for x_h, n_rows in ((xq, SC), (xk, kv_rows), (xv, kv_rows)):
                    for a in range(n_rows // 128):
                        nc.sync.dma_start(
                            out=x_h[a * 128 : (a + 1) * 128, :], in_=zt[:]
                        )

            for _rep in range(1 if loop_reps is None else loop_reps):
                emit_body()

    if split:
        _split_sync_waits(nc)
    return nc


_CACHE = {}
USE_CC = True


def _get_nc():
    if "nc" not in _CACHE:
        _CACHE["nc"] = _build_nc(use_cc=USE_CC)
    return _CACHE["nc"]


def _host_masks(h):
    """0/1 tiles for the <=2 boundary k-tiles of each local q-block."""
    ki = np.arange(128)[:, None]
    qi = np.arange(128)[None, :]
    m = np.zeros((NT, 2, 128, 128), dtype=np.float32)
    for t in range(NT):
        for j in range(2):
            kb = 2 * t + j          # global k-tile
            qb = 2 * t + h          # global q-block
            m[t, j] = (128 * kb + ki) <= (128 * qb + qi)
    return m.astype(ml_dtypes.bfloat16)


def kernel(**inputs):
    xq_full = np.asarray(inputs["inputs_for_queries"], dtype=np.float32)
    xk_full = np.asarray(inputs["inputs_for_keys"], dtype=np.float32)
    xv_full = np.asarray(inputs["inputs_for_values"], dtype=np.float32)
    wq = np.ascontiguousarray(np.asarray(inputs["Weight_Q"], dtype=np.float32))
    wk = np.ascontiguousarray(np.asarray(inputs["Weight_K"], dtype=np.float32))
    wv = np.ascontiguousarray(np.asarray(inputs["Weight_V"], dtype=np.float32))

    nc = _get_nc()

    masks_h = [_host_masks(h) for h in (0, 1)]
    in_maps = []
    for c in range(8):
        b, h = c // 2, c % 2
        rows = np.concatenate(
            [np.arange((2 * t + h) * 128, (2 * t + h + 1) * 128) for t in range(NT)]
        )
        if USE_CC:
            xk_c = xk_full[b][h * SC : (h + 1) * SC]
            xv_c = xv_full[b][h * SC : (h + 1) * SC]
        else:
            xk_c, xv_c = xk_full[b], xv_full[b]
        in_maps.append(
            {
                "xq": np.ascontiguousarray(xq_full[b][rows]),
                "xk": np.ascontiguousarray(xk_c),
                "xv": np.ascontiguousarray(xv_c),
                "wq": wq,
                "wk": wk,
                "wv": wv,
                "masks": masks_h[h],
                "vtag": np.zeros((1, KERNEL_UID), np.float32),
            }
        )

    trace = bool(int(os.environ.get("KERNEL_TRACE", "0")))
    res = run_bass_kernel_spmd(
        nc, in_maps, core_ids=list(range(8)), trace=trace
    )
    if trace:
        _CACHE["last_results"] = res

    out_full = np.empty((B, S, E), dtype=np.float32)
    for c in range(8):
        b, h = c // 2, c % 2
        oc = res.results[c]["out"]
        for t in range(NT):
            g = 2 * t + h
            out_full[b, g * 128 : (g + 1) * 128] = oc[t * 128 : (t + 1) * 128]
    return out_full

